# revision 53
# baseline (speedup 1.0000x reference)
"""Trainium2 Bass kernel for InterferenceBypassSelfAttention.

Math (reference):
  q_amp = softplus(x @ Wq_amp.T), k_amp = softplus(x @ Wk_amp.T)   (RMS-normed over head_dim)
  q_phi = pi*tanh(x @ Wq_phi.T),  k_phi = pi*tanh(x @ Wk_phi.T)
  scores = (q_amp*cos(q_phi))·(k_amp*cos(k_phi)) + (q_amp*sin(q_phi))·(k_amp*sin(k_phi))
         = <q_ext, k_ext> with q_ext = [q_amp*cos; q_amp*sin]  (128-dim stacked vector)
  out = softmax(causal(scores/8 * e^ls)) @ V @ Wo.T

Sharding: 16 heads / 8 cores -> 2 heads per core (both batches).  Each core:
  - projections for its 2 heads only (column-split weights), in transposed
    [out, token] layout so score matmuls need no transposes,
  - scores S^T[k,q] = Kx^T Qx via a single K=128 matmul (cos+sin fused by stacking),
  - softmax without max-subtraction (scores bounded by 8*e^ls, bias shift -8*e^ls),
  - AV via lhsT=[V|ones]: av[0:64]=exp@V (transposed), av[64]=denominator,
  - AllToAll reshard of head-outputs -> each core owns 256 tokens/batch,
  - full Wo matmul on its token shard.  Host just concatenates shards.
"""

import math
import sys

import numpy as np

for _p in ("/opt/trn_rl_repo", "/root/.axon_site/_ro/trn_rl_repo"):
    if _p not in sys.path:
        sys.path.append(_p)

import ml_dtypes

B, S, D, H, HD = 2, 2048, 1024, 16, 64
NCORES = 8
TOK = B * S            # 4096 tokens total
TSH = S // NCORES      # 256-token shard per batch per core after AllToAll
NT = S // 128          # 16 token tiles per batch
NQB = S // 512         # 4 q-blocks of 512 per batch

_BF = ml_dtypes.bfloat16


def _build_program():
    import concourse.bass as bass
    import concourse.mybir as mybir
    import concourse.tile as tile
    from concourse import bacc, hw_specs, masks

    fp32 = mybir.dt.float32
    bf16 = mybir.dt.bfloat16
    AF = mybir.ActivationFunctionType

    # Steer the act-table-load pass: it maps each function to the first
    # set containing it.  Strip our functions from every set except one
    # that covers the whole op group, so exp+ln (softplus) and
    # tanh+abs+sin (phases) each stay within a single table set.
    # Set ids (= positions in act_info.json) are preserved.
    if not getattr(hw_specs, "_ibsa_patched", False):
        _orig_gat = hw_specs.get_activation_tables
        _designated = {
            AF.Exp: "natural_log_exp_and_others",
            AF.Ln: "natural_log_exp_and_others",
            AF.Tanh: "silu_and_others",
            AF.Sin: "silu_and_others",
            AF.Abs: "silu_and_others",
            AF.Sqrt: "sqrt_and_others",
        }

        def _patched_gat(arch):
            tables = _orig_gat(arch)
            for name, funcs in tables.items():
                for f, target in _designated.items():
                    if name != target:
                        funcs.discard(f)
            return tables

        hw_specs.get_activation_tables = _patched_gat
        bacc.get_activation_tables = _patched_gat
        hw_specs._ibsa_patched = True

    nc = bacc.Bacc("TRN2", target_bir_lowering=False, debug=False, num_devices=NCORES)

    xT = nc.dram_tensor("xT", [D, TOK], bf16, kind="ExternalInput")
    wpT = nc.dram_tensor("wpT", [D, 512], bf16, kind="ExternalInput")
    wvT = nc.dram_tensor("wvT", [D, 128], bf16, kind="ExternalInput")
    woT = nc.dram_tensor("woT", [D, D], bf16, kind="ExternalInput")
    scb = nc.dram_tensor("scb", [128, 4], fp32, kind="ExternalInput")
    maskt = nc.dram_tensor("maskt", [128, 128], bf16, kind="ExternalInput")
    hb = nc.dram_tensor("hb", [128, 2], bf16, kind="ExternalInput")
    y = nc.dram_tensor("y", [B, 2, 128, D], fp32, kind="ExternalOutput")

    with tile.TileContext(nc) as tc:
        with (
            tc.tile_pool(name="singles", bufs=1) as singles,
            tc.tile_pool(name="per_bh", bufs=2) as per_bh,
            tc.tile_pool(name="work", bufs=2) as work,
            tc.tile_pool(name="xcp", bufs=4) as xcp,
            tc.tile_pool(name="exp", bufs=24) as expool,
            tc.tile_pool(name="small", bufs=2) as small,
            tc.tile_pool(name="ps5", bufs=3, space="PSUM") as ps5,
            tc.tile_pool(name="psB", bufs=2, space="PSUM") as psB,
            tc.tile_pool(name="psv", bufs=3, space="PSUM") as psv,
            tc.tile_pool(name="dram", bufs=2, space="DRAM") as dram,
            tc.tile_pool(name="dramsh", bufs=2, space="DRAM") as dramsh,
        ):
            # ---- constants / weights (loaded once) ----
            wp_sb = singles.tile([128, 8, 512], bf16, tag="wp")
            wv_sb = singles.tile([128, 8, 128], bf16, tag="wv")
            wo_sb = singles.tile([128, 8, 1024], bf16, tag="wo")
            for c in range(8):
                cp = slice(c * 128, (c + 1) * 128)
                nc.sync.dma_start(out=wp_sb[:, c, :], in_=wpT.ap()[cp, :])
            for c in range(8):
                cp = slice(c * 128, (c + 1) * 128)
                nc.sync.dma_start(out=wv_sb[:, c, :], in_=wvT.ap()[cp, :])
                nc.sync.dma_start(out=wo_sb[:, c, :], in_=woT.ap()[cp, :])
            ident_sb = singles.tile([128, 128], bf16, tag="ident")
            masks.make_identity(nc, ident_sb)
            scb_sb = singles.tile([128, 4], fp32, tag="scb")
            nc.sync.dma_start(out=scb_sb, in_=scb.ap())
            mask_sb = singles.tile([128, 128], bf16, tag="mask")
            nc.sync.dma_start(out=mask_sb, in_=maskt.ap())
            hb_sb = singles.tile([128, 2], bf16, tag="hb")
            nc.sync.dma_start(out=hb_sb, in_=hb.ap())
            eps_sb = singles.tile([128, 1], fp32, tag="eps")
            nc.vector.memset(eps_sb, 1e-6)
            pihalf_sb = singles.tile([128, 1], fp32, tag="pihalf")
            nc.vector.memset(pihalf_sb, math.pi / 2)

            a2a_tiles = []
            for b in range(B):
                # ---- per-batch persistent tiles ----
                qx = [per_bh.tile([128, S], bf16, tag=f"qx{h}", name=f"qx{h}") for h in range(2)]
                kx = [per_bh.tile([128, S], bf16, tag=f"kx{h}", name=f"kx{h}") for h in range(2)]
                vv = [per_bh.tile([128, NT, 65], bf16, tag=f"vv{h}", name=f"vv{h}") for h in range(2)]
                for h in range(2):
                    nc.vector.memset(vv[h][:, :, 64:65], 1.0)

                # ================= projections =================
                # Pass A: amplitudes (softplus) + RMS stats + V.  One ACT
                # table set (softplus) for the whole pass.
                amps = {}
                for blk in range(NQB):
                    tb = b * S + blk * 512
                    cs = slice(blk * 512, (blk + 1) * 512)
                    xc = xcp.tile([128, 8, 512], bf16, tag="xc")
                    for c in range(8):
                        nc.sync.dma_start(
                            out=xc[:, c, :],
                            in_=xT.ap()[c * 128 : (c + 1) * 128, tb : tb + 512],
                        )
                    for idx, nm in ((0, "amq"), (1, "amk")):
                        pp = ps5.tile([128, 512], fp32, tag="m5", name=f"pp{idx}")
                        for c in range(8):
                            nc.tensor.matmul(
                                pp,
                                lhsT=wp_sb[:, c, idx * 128 : (idx + 1) * 128],
                                rhs=xc[:, c, :],
                                start=(c == 0),
                                stop=(c == 7),
                            )
                        a = work.tile([128, 512], bf16, tag=f"{nm}{blk}",
                                      bufs=1, name=f"{nm}{blk}")
                        # softplus(z) = ln(exp(z) + 1); exp and ln share a table set
                        ez = work.tile([128, 512], fp32, tag="ez")
                        nc.scalar.activation(ez, pp, AF.Exp)
                        nc.scalar.activation(a, ez, AF.Ln, bias=1.0)
                        amps[(nm, blk)] = a
                        sq = work.tile([128, 512], bf16, tag="sq")
                        nc.vector.tensor_mul(sq, a, a)
                        ssq_ps = psv.tile([2, 512], fp32, tag="mv", name="ssq_ps")
                        nc.tensor.matmul(ssq_ps, lhsT=hb_sb, rhs=sq, start=True, stop=True)
                        # rstd = (ssq/64 + eps)^-1/2 = exp(-0.5*ln(v)) -- ln and
                        # exp are in the same ACT table set as the softplus ops,
                        # so the whole per-block chain needs no table switch
                        ssq_s = small.tile([2, 512], fp32, tag="ssq_s")
                        nc.vector.tensor_scalar(
                            ssq_s, ssq_ps, 1.0 / 64.0, 1e-6,
                            mybir.AluOpType.mult, mybir.AluOpType.add,
                        )
                        lnv = small.tile([2, 512], fp32, tag="lnv")
                        nc.scalar.activation(lnv, ssq_s, AF.Ln)
                        rstd = small.tile([2, 512], fp32, tag="rstd")
                        nc.scalar.activation(rstd, lnv, AF.Exp, scale=-0.5)
                        rb_d = dram.tile([2, 512], fp32, tag="rstd_d")
                        nc.sync.dma_start(out=rb_d, in_=rstd)
                        rb = work.tile([128, 512], fp32, tag="rb")
                        for h in range(2):
                            nc.sync.dma_start(
                                out=rb[h * 64 : (h + 1) * 64, :],
                                in_=bass.AP(
                                    tensor=rb_d.tensor,
                                    offset=rb_d.offset + h * 512,
                                    ap=[[0, 64], [1, 512]],
                                ),
                            )
                        nc.vector.tensor_mul(a, a, rb)

                    # V projection: transposed [o, t] with N=512, then PE
                    # transpose 128x128 tiles back to [t, o]
                    pv = ps5.tile([128, 512], fp32, tag="m5", name="pv")
                    for c in range(8):
                        nc.tensor.matmul(
                            pv,
                            lhsT=wv_sb[:, c, :],
                            rhs=xc[:, c, :],
                            start=(c == 0),
                            stop=(c == 7),
                        )
                    vt = work.tile([128, 512], bf16, tag="sq")
                    nc.vector.tensor_copy(vt, pv)
                    for tt in range(4):
                        tp = psv.tile([128, 128], bf16, tag="mv", name="tp")
                        nc.tensor.transpose(tp, vt[:, tt * 128 : (tt + 1) * 128], ident_sb)
                        for h in range(2):
                            nc.vector.tensor_copy(
                                vv[h][:, blk * 4 + tt, 0:64],
                                tp[:, h * 64 : (h + 1) * 64],
                            )

                # Pass B: phases (tanh+abs+sin all live in one ACT set) and
                # extended-vector build on GpSimd
                for blk in range(NQB):
                    tb = b * S + blk * 512
                    cs = slice(blk * 512, (blk + 1) * 512)
                    xc = xcp.tile([128, 8, 512], bf16, tag="xc")
                    for c in range(8):
                        nc.sync.dma_start(
                            out=xc[:, c, :],
                            in_=xT.ap()[c * 128 : (c + 1) * 128, tb : tb + 512],
                        )
                    phs = []
                    for idx in (2, 3):
                        pp = psB.tile([128, 512], fp32, tag="m5b", name=f"pp{idx}")
                        for c in range(8):
                            nc.tensor.matmul(
                                pp,
                                lhsT=wp_sb[:, c, idx * 128 : (idx + 1) * 128],
                                rhs=xc[:, c, :],
                                start=(c == 0),
                                stop=(c == 7),
                            )
                        ph = work.tile([128, 512], fp32, tag=f"act{idx}")
                        nc.scalar.activation(ph, pp, AF.Tanh)
                        phs.append(ph)
                    ph_q, ph_k = phs

                    # cos(pi*z) = sin(pi/2 - pi*|z|) keeps Sin input in [-pi, pi]
                    qa_abs = work.tile([128, 512], fp32, tag="qa_abs")
                    ka_abs = work.tile([128, 512], fp32, tag="ka_abs")
                    nc.scalar.activation(qa_abs, ph_q, AF.Abs)
                    nc.scalar.activation(ka_abs, ph_k, AF.Abs)
                    qc = work.tile([128, 512], bf16, tag="qc")
                    kc = work.tile([128, 512], bf16, tag="kc")
                    nc.scalar.activation(qc, qa_abs, AF.Sin, scale=-math.pi, bias=pihalf_sb)
                    nc.scalar.activation(kc, ka_abs, AF.Sin, scale=-math.pi, bias=pihalf_sb)
                    qs = work.tile([128, 512], bf16, tag="qs")
                    ks = work.tile([128, 512], bf16, tag="ks")
                    nc.scalar.activation(qs, ph_q, AF.Sin, scale=math.pi)
                    nc.scalar.activation(ks, ph_k, AF.Sin, scale=math.pi)

                    # extended vectors: rows 0:64 = amp*cos, rows 64:128 = amp*sin
                    amp_q = amps[("amq", blk)]
                    amp_k = amps[("amk", blk)]
                    for h in range(2):
                        hs = slice(h * 64, (h + 1) * 64)
                        nc.vector.tensor_mul(qx[h][0:64, cs], amp_q[hs, :], qc[hs, :])
                        nc.vector.tensor_mul(qx[h][64:128, cs], amp_q[hs, :], qs[hs, :])
                        nc.vector.tensor_mul(kx[h][0:64, cs], amp_k[hs, :], kc[hs, :])
                        nc.vector.tensor_mul(kx[h][64:128, cs], amp_k[hs, :], ks[hs, :])

                # ================= attention =================
                # Per half-batch [shard=t-tile, o_local, 128] AllToAll inputs;
                # each half's collective fires as soon as both heads finish it,
                # overlapping the rest of the attention.
                aot_h = [dram.tile([8, 128, 128], bf16, tag=f"aot{p}", name=f"aot{p}")
                         for p in range(2)]
                for qb in range(NQB):
                    for h in range(2):
                        ex_tiles = []
                        for kt in range(4 * qb + 4):
                            diag = kt >= 4 * qb
                            q_lo = kt * 128 if diag else qb * 512
                            span = (qb + 1) * 512 - q_lo
                            sc = ps5.tile([128, 512], fp32, tag="m5")
                            nc.tensor.matmul(
                                sc[:, :span],
                                lhsT=kx[h][:, kt * 128 : (kt + 1) * 128],
                                rhs=qx[h][:, q_lo : q_lo + span],
                                start=True,
                                stop=True,
                            )
                            ex = expool.tile([128, 512], bf16, tag="ex")
                            nc.scalar.activation(
                                ex[:, :span],
                                sc[:, :span],
                                AF.Exp,
                                scale=scb_sb[:, h : h + 1],
                                bias=scb_sb[:, 2 + h : 3 + h],
                            )
                            if diag:
                                nc.vector.tensor_mul(ex[:, 0:128], ex[:, 0:128], mask_sb)
                            ex_tiles.append((q_lo - qb * 512, span, ex))

                        av = psv.tile([65, 512], fp32, tag="mv")
                        last = len(ex_tiles) - 1
                        for kt, (off, span, ex) in enumerate(ex_tiles):
                            nc.tensor.matmul(
                                av[:, off : off + span],
                                lhsT=vv[h][:, kt, :],
                                rhs=ex[:, :span],
                                start=(kt == 0),
                                stop=(kt == last),
                            )
                        r0 = small.tile([1, 512], fp32, tag="r0")
                        nc.vector.tensor_copy(r0, av[64:65, :])
                        r1 = small.tile([1, 512], fp32, tag="r1")
                        nc.vector.reciprocal_approx_fast(r1, r0)
                        r1_d = dram.tile([1, 512], fp32, tag="r1_d")
                        nc.sync.dma_start(out=r1_d, in_=r1)
                        rq = work.tile([64, 512], fp32, tag="rq")
                        nc.sync.dma_start(
                            out=rq,
                            in_=bass.AP(
                                tensor=r1_d.tensor,
                                offset=r1_d.offset,
                                ap=[[0, 64], [1, 512]],
                            ),
                        )
                        aoT = work.tile([64, 512], bf16, tag="aoT")
                        nc.vector.tensor_mul(aoT, av[0:64, :], rq)
                        p, jbase = qb // 2, (qb % 2) * 4
                        for st in range(4):
                            nc.sync.dma_start(
                                out=aot_h[p][jbase + st, h * 64 : (h + 1) * 64, :],
                                in_=aoT[:, st * 128 : (st + 1) * 128],
                            )

                    # fire each half's AllToAll as soon as it is complete
                    if qb % 2 == 1:
                        p = qb // 2
                        a2a = dramsh.tile([8, 128, 128], bf16, tag=f"a2a{p}",
                                          name=f"a2a{b}{p}")
                        nc.gpsimd.collective_compute(
                            "AllToAll",
                            mybir.AluOpType.bypass,
                            replica_groups=[list(range(NCORES))],
                            ins=[aot_h[p][:]],
                            outs=[a2a[:]],
                        )
                        a2a_tiles.append((b, p, a2a))

            # ================= output projection =============
            for b, p, a2a in a2a_tiles:
                lh = []
                for src in range(8):
                    lt = work.tile([128, 128], bf16, tag=f"lh{src}", name=f"lh{src}")
                    nc.sync.dma_start(out=lt, in_=a2a[src])
                    lh.append(lt)
                for nb in range(2):
                    yp = ps5.tile([128, 512], fp32, tag="m5")
                    for src in range(8):
                        nc.tensor.matmul(
                            yp,
                            lhsT=lh[src],
                            rhs=wo_sb[:, src, nb * 512 : (nb + 1) * 512],
                            start=(src == 0),
                            stop=(src == 7),
                        )
                    ys = work.tile([128, 512], fp32, tag="ys")
                    nc.vector.tensor_copy(ys, yp)
                    nc.sync.dma_start(
                        out=y.ap()[b, p, :, nb * 512 : (nb + 1) * 512],
                        in_=ys,
                    )
    nc.compile()
    return nc


_NC_CACHE = None


def _get_program():
    global _NC_CACHE
    if _NC_CACHE is None:
        _NC_CACHE = _build_program()
    return _NC_CACHE


def make_in_maps(x, Wq_amp, Wk_amp, Wq_phi, Wk_phi, Wv, Wo, score_log_scale):
    x = np.asarray(x, np.float32)
    xT = np.ascontiguousarray(x.reshape(TOK, D).T).astype(_BF)
    woT = np.ascontiguousarray(np.asarray(Wo, np.float32).T).astype(_BF)
    k_idx, q_idx = np.meshgrid(np.arange(128), np.arange(128), indexing="ij")
    maskt = (k_idx <= q_idx).astype(_BF)
    hb = np.zeros((128, 2), _BF)
    hb[0:64, 0] = 1
    hb[64:128, 1] = 1
    ls = np.asarray(score_log_scale, np.float32)

    in_maps = []
    for c in range(NCORES):
        hs = slice(c * 128, (c + 1) * 128)
        wpT = np.concatenate(
            [
                np.asarray(Wq_amp, np.float32)[hs].T,
                np.asarray(Wk_amp, np.float32)[hs].T,
                np.asarray(Wq_phi, np.float32)[hs].T,
                np.asarray(Wk_phi, np.float32)[hs].T,
            ],
            axis=1,
        ).astype(_BF)
        wvT = np.ascontiguousarray(np.asarray(Wv, np.float32)[hs].T).astype(_BF)
        e0, e1 = math.exp(ls[2 * c]), math.exp(ls[2 * c + 1])
        scbv = np.tile(
            np.array([e0 / 8.0, e1 / 8.0, -8.0 * e0, -8.0 * e1], np.float32), (128, 1)
        )
        in_maps.append(
            {
                "xT": xT,
                "wpT": np.ascontiguousarray(wpT),
                "wvT": wvT,
                "woT": woT,
                "scb": scbv,
                "maskt": maskt,
                "hb": hb,
            }
        )
    return in_maps


def gather_output(results):
    Y = np.empty((B, S, D), np.float32)
    for c in range(NCORES):
        yc = results[c]["y"]  # [B, 2, 128, D]
        for p in range(2):
            lo = p * 1024 + c * 128
            Y[:, lo : lo + 128, :] = yc[:, p]
    return Y


def kernel(x, Wq_amp, Wk_amp, Wq_phi, Wk_phi, Wv, Wo, score_log_scale):
    from concourse.bass_utils import run_bass_kernel_spmd

    nc = _get_program()
    in_maps = make_in_maps(x, Wq_amp, Wk_amp, Wq_phi, Wk_phi, Wv, Wo, score_log_scale)
    res = run_bass_kernel_spmd(nc, in_maps, list(range(NCORES)))
    return gather_output(res.results)


# revision 56
# speedup vs baseline: 1.1989x; 1.1989x over previous
"""Trainium2 Bass kernel for InterferenceBypassSelfAttention.

Math (reference):
  q_amp = softplus(x @ Wq_amp.T), k_amp = softplus(x @ Wk_amp.T)   (RMS-normed over head_dim)
  q_phi = pi*tanh(x @ Wq_phi.T),  k_phi = pi*tanh(x @ Wk_phi.T)
  scores = (q_amp*cos(q_phi))·(k_amp*cos(k_phi)) + (q_amp*sin(q_phi))·(k_amp*sin(k_phi))
         = <q_ext, k_ext> with q_ext = [q_amp*cos; q_amp*sin]  (128-dim stacked vector)
  out = softmax(causal(scores/8 * e^ls)) @ V @ Wo.T

Sharding: 16 heads / 8 cores -> 2 heads per core (both batches).  Each core:
  - projections for its 2 heads only (column-split weights), in transposed
    [out, token] layout so score matmuls need no transposes,
  - scores S^T[k,q] = Kx^T Qx via a single K=128 matmul (cos+sin fused by stacking),
  - softmax without max-subtraction (scores bounded by 8*e^ls, bias shift -8*e^ls),
  - AV via lhsT=[V|ones]: av[0:64]=exp@V (transposed), av[64]=denominator,
  - AllToAll reshard of head-outputs -> each core owns 256 tokens/batch,
  - full Wo matmul on its token shard.  Host just concatenates shards.
"""

import math
import sys

import numpy as np

for _p in ("/opt/trn_rl_repo", "/root/.axon_site/_ro/trn_rl_repo"):
    if _p not in sys.path:
        sys.path.append(_p)

import ml_dtypes

B, S, D, H, HD = 2, 2048, 1024, 16, 64
NCORES = 8
TOK = B * S            # 4096 tokens total
TSH = S // NCORES      # 256-token shard per batch per core after AllToAll
NT = S // 128          # 16 token tiles per batch
NQB = S // 512         # 4 q-blocks of 512 per batch

_BF = ml_dtypes.bfloat16


def _build_program():
    import concourse.bass as bass
    import concourse.mybir as mybir
    import concourse.tile as tile
    from concourse import bacc, hw_specs, masks

    fp32 = mybir.dt.float32
    bf16 = mybir.dt.bfloat16
    AF = mybir.ActivationFunctionType

    # Steer the act-table-load pass: it maps each function to the first
    # set containing it.  Strip our functions from every set except one
    # that covers the whole op group, so exp+ln (softplus) and
    # tanh+abs+sin (phases) each stay within a single table set.
    # Set ids (= positions in act_info.json) are preserved.
    if not getattr(hw_specs, "_ibsa_patched", False):
        _orig_gat = hw_specs.get_activation_tables
        _designated = {
            AF.Exp: "natural_log_exp_and_others",
            AF.Ln: "natural_log_exp_and_others",
            AF.Tanh: "silu_and_others",
            AF.Sin: "silu_and_others",
            AF.Abs: "silu_and_others",
            AF.Sqrt: "sqrt_and_others",
        }

        def _patched_gat(arch):
            tables = _orig_gat(arch)
            for name, funcs in tables.items():
                for f, target in _designated.items():
                    if name != target:
                        funcs.discard(f)
            return tables

        hw_specs.get_activation_tables = _patched_gat
        bacc.get_activation_tables = _patched_gat
        hw_specs._ibsa_patched = True

    nc = bacc.Bacc("TRN2", target_bir_lowering=False, debug=False, num_devices=NCORES)

    xT = nc.dram_tensor("xT", [D, TOK], bf16, kind="ExternalInput")
    wpT = nc.dram_tensor("wpT", [D, 512], bf16, kind="ExternalInput")
    wvT = nc.dram_tensor("wvT", [D, 128], bf16, kind="ExternalInput")
    woT = nc.dram_tensor("woT", [D, D], bf16, kind="ExternalInput")
    scb = nc.dram_tensor("scb", [128, 4], fp32, kind="ExternalInput")
    maskt = nc.dram_tensor("maskt", [128, 128], bf16, kind="ExternalInput")
    hb = nc.dram_tensor("hb", [128, 2], bf16, kind="ExternalInput")
    y = nc.dram_tensor("y", [B, 2, 128, D], fp32, kind="ExternalOutput")

    with tile.TileContext(nc) as tc:
        with (
            tc.tile_pool(name="singles", bufs=1) as singles,
            tc.tile_pool(name="per_bh", bufs=2) as per_bh,
            tc.tile_pool(name="work", bufs=2) as work,
            tc.tile_pool(name="xcp", bufs=3) as xcp,
            tc.tile_pool(name="exp", bufs=24) as expool,
            tc.tile_pool(name="small", bufs=2) as small,
            tc.tile_pool(name="ps5", bufs=4, space="PSUM") as ps5,
            tc.tile_pool(name="psv", bufs=3, space="PSUM") as psv,
            tc.tile_pool(name="dram", bufs=2, space="DRAM") as dram,
            tc.tile_pool(name="dramsh", bufs=2, space="DRAM") as dramsh,
        ):
            # ---- constants / weights (loaded once) ----
            wp_sb = singles.tile([128, 8, 512], bf16, tag="wp")
            wv_sb = singles.tile([128, 8, 128], bf16, tag="wv")
            wo_sb = singles.tile([128, 8, 1024], bf16, tag="wo")
            for c in range(8):
                cp = slice(c * 128, (c + 1) * 128)
                nc.sync.dma_start(out=wp_sb[:, c, :], in_=wpT.ap()[cp, :])
            for c in range(8):
                cp = slice(c * 128, (c + 1) * 128)
                nc.sync.dma_start(out=wv_sb[:, c, :], in_=wvT.ap()[cp, :])
                nc.sync.dma_start(out=wo_sb[:, c, :], in_=woT.ap()[cp, :])
            ident_sb = singles.tile([128, 128], bf16, tag="ident")
            masks.make_identity(nc, ident_sb)
            scb_sb = singles.tile([128, 4], fp32, tag="scb")
            nc.sync.dma_start(out=scb_sb, in_=scb.ap())
            mask_sb = singles.tile([128, 128], bf16, tag="mask")
            nc.sync.dma_start(out=mask_sb, in_=maskt.ap())
            hb_sb = singles.tile([128, 2], bf16, tag="hb")
            nc.sync.dma_start(out=hb_sb, in_=hb.ap())
            eps_sb = singles.tile([128, 1], fp32, tag="eps")
            nc.vector.memset(eps_sb, 1e-6)
            pihalf_sb = singles.tile([128, 1], fp32, tag="pihalf")
            nc.vector.memset(pihalf_sb, math.pi / 2)

            a2a_tiles = []
            for b in range(B):
                # ---- per-batch persistent tiles ----
                qx = [per_bh.tile([128, S], bf16, tag=f"qx{h}", name=f"qx{h}") for h in range(2)]
                kx = [per_bh.tile([128, S], bf16, tag=f"kx{h}", name=f"kx{h}") for h in range(2)]
                vv = [per_bh.tile([128, NT, 65], bf16, tag=f"vv{h}", name=f"vv{h}") for h in range(2)]
                for h in range(2):
                    nc.vector.memset(vv[h][:, :, 64:65], 1.0)

                # ================= projections =================
                # Pass A: amplitudes (softplus) + RMS stats + V.  One ACT
                # table set (softplus) for the whole pass.
                amps = {}
                for blk in range(NQB):
                    tb = b * S + blk * 512
                    cs = slice(blk * 512, (blk + 1) * 512)
                    xc = xcp.tile([128, 8, 512], bf16, tag="xc")
                    for c in range(8):
                        nc.sync.dma_start(
                            out=xc[:, c, :],
                            in_=xT.ap()[c * 128 : (c + 1) * 128, tb : tb + 512],
                        )
                    for idx, nm in ((0, "amq"), (1, "amk")):
                        pp = ps5.tile([128, 512], fp32, tag="m5", name=f"pp{idx}")
                        for c in range(8):
                            nc.tensor.matmul(
                                pp,
                                lhsT=wp_sb[:, c, idx * 128 : (idx + 1) * 128],
                                rhs=xc[:, c, :],
                                start=(c == 0),
                                stop=(c == 7),
                            )
                        a = work.tile([128, 512], bf16, tag=f"{nm}{blk}",
                                      bufs=1, name=f"{nm}{blk}")
                        # softplus(z) = ln(exp(z) + 1); exp and ln share a table set
                        ez = work.tile([128, 512], fp32, tag="ez")
                        nc.scalar.activation(ez, pp, AF.Exp)
                        nc.scalar.activation(a, ez, AF.Ln, bias=1.0)
                        amps[(nm, blk)] = a
                        sq = work.tile([128, 512], bf16, tag="sq")
                        nc.vector.tensor_mul(sq, a, a)
                        ssq_ps = psv.tile([2, 512], fp32, tag="mv", name="ssq_ps")
                        nc.tensor.matmul(ssq_ps, lhsT=hb_sb, rhs=sq, start=True, stop=True)
                        # rstd = (ssq/64 + eps)^-1/2 = exp(-0.5*ln(v)) -- ln and
                        # exp are in the same ACT table set as the softplus ops,
                        # so the whole per-block chain needs no table switch
                        ssq_s = small.tile([2, 512], fp32, tag="ssq_s")
                        nc.vector.tensor_scalar(
                            ssq_s, ssq_ps, 1.0 / 64.0, 1e-6,
                            mybir.AluOpType.mult, mybir.AluOpType.add,
                        )
                        lnv = small.tile([2, 512], fp32, tag="lnv")
                        nc.scalar.activation(lnv, ssq_s, AF.Ln)
                        rstd = small.tile([2, 512], fp32, tag="rstd")
                        nc.scalar.activation(rstd, lnv, AF.Exp, scale=-0.5)
                        rb_d = dram.tile([2, 512], fp32, tag="rstd_d")
                        nc.sync.dma_start(out=rb_d, in_=rstd)
                        rb = work.tile([128, 512], fp32, tag="rb")
                        for h in range(2):
                            nc.sync.dma_start(
                                out=rb[h * 64 : (h + 1) * 64, :],
                                in_=bass.AP(
                                    tensor=rb_d.tensor,
                                    offset=rb_d.offset + h * 512,
                                    ap=[[0, 64], [1, 512]],
                                ),
                            )
                        nc.vector.tensor_mul(a, a, rb)

                    # V projection: transposed [o, t] with N=512, then PE
                    # transpose 128x128 tiles back to [t, o]
                    pv = ps5.tile([128, 512], fp32, tag="m5", name="pv")
                    for c in range(8):
                        nc.tensor.matmul(
                            pv,
                            lhsT=wv_sb[:, c, :],
                            rhs=xc[:, c, :],
                            start=(c == 0),
                            stop=(c == 7),
                        )
                    vt = work.tile([128, 512], bf16, tag="sq")
                    nc.vector.tensor_copy(vt, pv)
                    for tt in range(4):
                        tp = psv.tile([128, 128], bf16, tag="mv", name="tp")
                        nc.tensor.transpose(tp, vt[:, tt * 128 : (tt + 1) * 128], ident_sb)
                        for h in range(2):
                            nc.vector.tensor_copy(
                                vv[h][:, blk * 4 + tt, 0:64],
                                tp[:, h * 64 : (h + 1) * 64],
                            )

                # Pass B: phases (tanh+abs+sin all live in one ACT set) and
                # extended-vector build on GpSimd
                for blk in range(NQB):
                    tb = b * S + blk * 512
                    cs = slice(blk * 512, (blk + 1) * 512)
                    xc = xcp.tile([128, 8, 512], bf16, tag="xc")
                    for c in range(8):
                        nc.sync.dma_start(
                            out=xc[:, c, :],
                            in_=xT.ap()[c * 128 : (c + 1) * 128, tb : tb + 512],
                        )
                    phs = []
                    for idx in (2, 3):
                        pp = ps5.tile([128, 512], fp32, tag="m5", name=f"pp{idx}")
                        for c in range(8):
                            nc.tensor.matmul(
                                pp,
                                lhsT=wp_sb[:, c, idx * 128 : (idx + 1) * 128],
                                rhs=xc[:, c, :],
                                start=(c == 0),
                                stop=(c == 7),
                            )
                        ph = work.tile([128, 512], fp32, tag=f"act{idx}")
                        nc.scalar.activation(ph, pp, AF.Tanh)
                        phs.append(ph)
                    ph_q, ph_k = phs

                    # cos(pi*z) = sin(pi/2 - pi*|z|) keeps Sin input in [-pi, pi]
                    qa_abs = work.tile([128, 512], fp32, tag="qa_abs")
                    ka_abs = work.tile([128, 512], fp32, tag="ka_abs")
                    nc.scalar.activation(qa_abs, ph_q, AF.Abs)
                    nc.scalar.activation(ka_abs, ph_k, AF.Abs)
                    qc = work.tile([128, 512], bf16, tag="qc")
                    kc = work.tile([128, 512], bf16, tag="kc")
                    nc.scalar.activation(qc, qa_abs, AF.Sin, scale=-math.pi, bias=pihalf_sb)
                    nc.scalar.activation(kc, ka_abs, AF.Sin, scale=-math.pi, bias=pihalf_sb)
                    qs = work.tile([128, 512], bf16, tag="qs")
                    ks = work.tile([128, 512], bf16, tag="ks")
                    nc.scalar.activation(qs, ph_q, AF.Sin, scale=math.pi)
                    nc.scalar.activation(ks, ph_k, AF.Sin, scale=math.pi)

                    # extended vectors: rows 0:64 = amp*cos, rows 64:128 = amp*sin
                    amp_q = amps[("amq", blk)]
                    amp_k = amps[("amk", blk)]
                    for h in range(2):
                        hs = slice(h * 64, (h + 1) * 64)
                        nc.vector.tensor_mul(qx[h][0:64, cs], amp_q[hs, :], qc[hs, :])
                        nc.vector.tensor_mul(qx[h][64:128, cs], amp_q[hs, :], qs[hs, :])
                        nc.vector.tensor_mul(kx[h][0:64, cs], amp_k[hs, :], kc[hs, :])
                        nc.vector.tensor_mul(kx[h][64:128, cs], amp_k[hs, :], ks[hs, :])

                # ================= attention =================
                # Per half-batch [shard=t-tile, o_local, 128] AllToAll inputs;
                # each half's collective fires as soon as both heads finish it,
                # overlapping the rest of the attention.
                aot_h = [dram.tile([8, 128, 128], bf16, tag=f"aot{p}", name=f"aot{p}")
                         for p in range(2)]
                for qb in range(NQB):
                    for h in range(2):
                        ex_tiles = []
                        for kt in range(4 * qb + 4):
                            diag = kt >= 4 * qb
                            q_lo = kt * 128 if diag else qb * 512
                            span = (qb + 1) * 512 - q_lo
                            sc = ps5.tile([128, 512], fp32, tag="m5")
                            nc.tensor.matmul(
                                sc[:, :span],
                                lhsT=kx[h][:, kt * 128 : (kt + 1) * 128],
                                rhs=qx[h][:, q_lo : q_lo + span],
                                start=True,
                                stop=True,
                            )
                            ex = expool.tile([128, 512], bf16, tag="ex")
                            nc.scalar.activation(
                                ex[:, :span],
                                sc[:, :span],
                                AF.Exp,
                                scale=scb_sb[:, h : h + 1],
                                bias=scb_sb[:, 2 + h : 3 + h],
                            )
                            if diag:
                                nc.vector.tensor_mul(ex[:, 0:128], ex[:, 0:128], mask_sb)
                            ex_tiles.append((q_lo - qb * 512, span, ex))

                        av = psv.tile([65, 512], fp32, tag="mv")
                        last = len(ex_tiles) - 1
                        for kt, (off, span, ex) in enumerate(ex_tiles):
                            nc.tensor.matmul(
                                av[:, off : off + span],
                                lhsT=vv[h][:, kt, :],
                                rhs=ex[:, :span],
                                start=(kt == 0),
                                stop=(kt == last),
                            )
                        r0 = small.tile([1, 512], fp32, tag="r0")
                        nc.vector.tensor_copy(r0, av[64:65, :])
                        r1 = small.tile([1, 512], fp32, tag="r1")
                        nc.vector.reciprocal_approx_fast(r1, r0)
                        r1_d = dram.tile([1, 512], fp32, tag="r1_d")
                        nc.sync.dma_start(out=r1_d, in_=r1)
                        rq = work.tile([64, 512], fp32, tag="rq")
                        nc.sync.dma_start(
                            out=rq,
                            in_=bass.AP(
                                tensor=r1_d.tensor,
                                offset=r1_d.offset,
                                ap=[[0, 64], [1, 512]],
                            ),
                        )
                        aoT = work.tile([64, 512], bf16, tag="aoT")
                        nc.vector.tensor_mul(aoT, av[0:64, :], rq)
                        p, jbase = qb // 2, (qb % 2) * 4
                        for st in range(4):
                            nc.sync.dma_start(
                                out=aot_h[p][jbase + st, h * 64 : (h + 1) * 64, :],
                                in_=aoT[:, st * 128 : (st + 1) * 128],
                            )

                    # fire each half's AllToAll as soon as it is complete
                    if qb % 2 == 1:
                        p = qb // 2
                        a2a = dramsh.tile([8, 128, 128], bf16, tag=f"a2a{p}",
                                          name=f"a2a{b}{p}")
                        nc.gpsimd.collective_compute(
                            "AllToAll",
                            mybir.AluOpType.bypass,
                            replica_groups=[list(range(NCORES))],
                            ins=[aot_h[p][:]],
                            outs=[a2a[:]],
                        )
                        a2a_tiles.append((b, p, a2a))

            # ================= output projection =============
            for b, p, a2a in a2a_tiles:
                lh = []
                for src in range(8):
                    lt = work.tile([128, 128], bf16, tag=f"lh{src}", name=f"lh{src}")
                    nc.sync.dma_start(out=lt, in_=a2a[src])
                    lh.append(lt)
                for nb in range(2):
                    yp = ps5.tile([128, 512], fp32, tag="m5")
                    for src in range(8):
                        nc.tensor.matmul(
                            yp,
                            lhsT=lh[src],
                            rhs=wo_sb[:, src, nb * 512 : (nb + 1) * 512],
                            start=(src == 0),
                            stop=(src == 7),
                        )
                    ys = work.tile([128, 512], fp32, tag="ys")
                    nc.vector.tensor_copy(ys, yp)
                    nc.sync.dma_start(
                        out=y.ap()[b, p, :, nb * 512 : (nb + 1) * 512],
                        in_=ys,
                    )
    nc.compile()
    return nc


_NC_CACHE = None


def _get_program():
    global _NC_CACHE
    if _NC_CACHE is None:
        _NC_CACHE = _build_program()
    return _NC_CACHE


def make_in_maps(x, Wq_amp, Wk_amp, Wq_phi, Wk_phi, Wv, Wo, score_log_scale):
    x = np.asarray(x, np.float32)
    xT = np.ascontiguousarray(x.reshape(TOK, D).T).astype(_BF)
    woT = np.ascontiguousarray(np.asarray(Wo, np.float32).T).astype(_BF)
    k_idx, q_idx = np.meshgrid(np.arange(128), np.arange(128), indexing="ij")
    maskt = (k_idx <= q_idx).astype(_BF)
    hb = np.zeros((128, 2), _BF)
    hb[0:64, 0] = 1
    hb[64:128, 1] = 1
    ls = np.asarray(score_log_scale, np.float32)

    in_maps = []
    for c in range(NCORES):
        hs = slice(c * 128, (c + 1) * 128)
        wpT = np.concatenate(
            [
                np.asarray(Wq_amp, np.float32)[hs].T,
                np.asarray(Wk_amp, np.float32)[hs].T,
                np.asarray(Wq_phi, np.float32)[hs].T,
                np.asarray(Wk_phi, np.float32)[hs].T,
            ],
            axis=1,
        ).astype(_BF)
        wvT = np.ascontiguousarray(np.asarray(Wv, np.float32)[hs].T).astype(_BF)
        e0, e1 = math.exp(ls[2 * c]), math.exp(ls[2 * c + 1])
        scbv = np.tile(
            np.array([e0 / 8.0, e1 / 8.0, -8.0 * e0, -8.0 * e1], np.float32), (128, 1)
        )
        in_maps.append(
            {
                "xT": xT,
                "wpT": np.ascontiguousarray(wpT),
                "wvT": wvT,
                "woT": woT,
                "scb": scbv,
                "maskt": maskt,
                "hb": hb,
            }
        )
    return in_maps


def gather_output(results):
    Y = np.empty((B, S, D), np.float32)
    for c in range(NCORES):
        yc = results[c]["y"]  # [B, 2, 128, D]
        for p in range(2):
            lo = p * 1024 + c * 128
            Y[:, lo : lo + 128, :] = yc[:, p]
    return Y


def kernel(x, Wq_amp, Wk_amp, Wq_phi, Wk_phi, Wv, Wo, score_log_scale):
    from concourse.bass_utils import run_bass_kernel_spmd

    nc = _get_program()
    in_maps = make_in_maps(x, Wq_amp, Wk_amp, Wq_phi, Wk_phi, Wv, Wo, score_log_scale)
    res = run_bass_kernel_spmd(nc, in_maps, list(range(NCORES)))
    return gather_output(res.results)
